# revision 6
# baseline (speedup 1.0000x reference)
"""Trainium2 Bass kernel for nn_Attention (B=4, S=1024, E=1024, H=16, D=64).

Sharding: 8 cores = 4 batches x 2 head-groups (8 heads each, host-side
slicing, no collectives). Per core: QKV projections in float32r on the PE
(activations PE-transposed on-chip so the contraction dim sits on
partitions), S^T = K^T-chunk^T-style matmuls with two heads row-packed in
the 128x128 array, exp on ACT with the 1/sqrt(D) scale folded in and the
additive attention mask folded into V-row scaling (exp(s+m) =
exp(s)*exp(m)), PV accumulation with an extra ones-column in V producing
the softmax denominator for free, PE-transpose back, per-partition
normalize, DMA out.
"""

import os
import sys

import numpy as np

for _p in ("/opt/trn_rl_repo",):
    if _p not in sys.path:
        sys.path.insert(0, _p)

B, SQ, SKV, E, H = 4, 1024, 1024, 1024, 16
D = 64
HPC = 8            # heads per core
EO = HPC * D       # 512 per-core projection output dim
N_CORES = 8
EI = E // 128      # 8 contraction chunks
KC = SKV // 128    # 8 key chunks
OC = EO // 128     # 4 output chunks = 4 head pairs

_NC_CACHE = {}
LAST_RESULTS = None


def _build_nc():
    import concourse.mybir as mybir
    from concourse import bacc
    from concourse.masks import make_identity
    from concourse.tile import TileContext

    f32 = mybir.dt.float32
    f32r = mybir.dt.float32r
    AF = mybir.ActivationFunctionType

    nc = bacc.Bacc("TRN2", target_bir_lowering=False, debug=False)
    hid = nc.declare_dram_parameter("hid", [SQ, E], f32, isOutput=False)
    ctx = nc.declare_dram_parameter("ctx", [SKV, E], f32, isOutput=False)
    wq = nc.declare_dram_parameter("wq", [E, EO], f32, isOutput=False)
    wk = nc.declare_dram_parameter("wk", [E, EO], f32, isOutput=False)
    wv = nc.declare_dram_parameter("wv", [E, EO], f32, isOutput=False)
    bq = nc.declare_dram_parameter("bq", [EO], f32, isOutput=False)
    bk = nc.declare_dram_parameter("bk", [EO], f32, isOutput=False)
    bv = nc.declare_dram_parameter("bv", [EO], f32, isOutput=False)
    mask = nc.declare_dram_parameter("mask", [SKV], f32, isOutput=False)
    out = nc.declare_dram_parameter("out", [SQ, EO], f32, isOutput=True)

    r = lambda ap: ap.bitcast(f32r)

    with TileContext(nc) as tc:
        with (
            tc.tile_pool(name="const", bufs=1) as p_const,
            tc.tile_pool(name="w", bufs=1) as p_w,
            tc.tile_pool(name="actT", bufs=EI) as p_actT,
            tc.tile_pool(name="pers", bufs=1) as p_pers,
            tc.tile_pool(name="nat", bufs=3) as p_nat,
            tc.tile_pool(name="pt", bufs=3) as p_pt,
            tc.tile_pool(name="ot", bufs=4) as p_ot,
            tc.tile_pool(name="small", bufs=8) as p_small,
            tc.tile_pool(name="outp", bufs=10) as p_out,
            tc.tile_pool(name="ps_a", bufs=2, space="PSUM") as ps_a,
            tc.tile_pool(name="ps_s", bufs=2, space="PSUM") as ps_s,
            tc.tile_pool(name="ps_pv", bufs=2, space="PSUM") as ps_pv,
        ):
            ident = p_const.tile([128, 128], f32, name="ident")
            make_identity(nc, ident)

            bq_sb = p_const.tile([128, OC], f32, name="bq_sb")
            nc.sync.dma_start(out=bq_sb, in_=bq.rearrange("(c p) -> p c", p=128))
            bk_sb = p_const.tile([128, OC], f32, name="bk_sb")
            nc.sync.dma_start(out=bk_sb, in_=bk.rearrange("(c p) -> p c", p=128))
            bv_sb = p_const.tile([1, EO], f32, name="bv_sb")
            nc.sync.dma_start(out=bv_sb, in_=bv[None, :])
            mask_sb = p_const.tile([128, KC], f32, name="mask_sb")
            nc.sync.dma_start(out=mask_sb, in_=mask.rearrange("(c p) -> p c", p=128))
            wexp = p_const.tile([128, KC], f32, name="wexp")
            nc.scalar.activation(wexp, mask_sb, AF.Exp)
            ones1 = p_const.tile([1, 128], f32, name="ones1")
            nc.vector.memset(ones1, 1.0)
            # broadcast bv across partitions: ones1.T @ bv_sb
            bvps = ps_a.tile([128, EO], f32, name="bvps", tag="a")
            nc.tensor.matmul(bvps, lhsT=ones1, rhs=bv_sb, start=True, stop=True)
            bv_bc = p_const.tile([128, EO], f32, name="bv_bc")
            nc.vector.tensor_copy(bv_bc, bvps)

            wq_sb = []
            wk_sb = []
            wv_sb = []
            for e in range(EI):
                t = p_w.tile([128, EO], f32r, name=f"wq_sb{e}", tag=f"wq{e}")
                nc.sync.dma_start(out=t, in_=r(wq[e * 128:(e + 1) * 128, :]))
                wq_sb.append(t)
            for e in range(EI):
                t = p_w.tile([128, EO], f32r, name=f"wk_sb{e}", tag=f"wk{e}")
                nc.sync.dma_start(out=t, in_=r(wk[e * 128:(e + 1) * 128, :]))
                wk_sb.append(t)
            for e in range(EI):
                t = p_w.tile([128, EO], f32r, name=f"wv_sb{e}", tag=f"wv{e}")
                nc.sync.dma_start(out=t, in_=r(wv[e * 128:(e + 1) * 128, :]))
                wv_sb.append(t)

            # ---- hidden transposes: hidT[e] = hid^T chunk [128 e, SQ q] ----
            hidT = [
                p_actT.tile([128, SQ], f32r, name=f"hidT{e}", tag="actT")
                for e in range(EI)
            ]
            for s in range(SQ // 128):
                hnat = p_nat.tile([128, E], f32, name="hnat", tag="nat")
                nc.sync.dma_start(out=hnat, in_=hid[s * 128:(s + 1) * 128, :])
                for e in range(EI):
                    tps = ps_a.tile([128, 128], f32, name="tps", tag="a")
                    nc.tensor.transpose(tps, hnat[:, e * 128:(e + 1) * 128], ident)
                    nc.vector.tensor_copy(hidT[e][:, s * 128:(s + 1) * 128], tps)

            # ---- Q^T = (hid @ wq)^T + bias: QT[o] [128 eo, SQ q] ----
            QT = [p_pers.tile([128, SQ], f32r, name=f"QT{o}") for o in range(OC)]
            for o in range(OC):
                for hlf in range(2):
                    pq = ps_a.tile([128, 512], f32, name="pq", tag="a")
                    for e in range(EI):
                        nc.tensor.matmul(
                            pq,
                            lhsT=wq_sb[e][:, o * 128:(o + 1) * 128],
                            rhs=hidT[e][:, hlf * 512:(hlf + 1) * 512],
                            start=(e == 0),
                            stop=(e == EI - 1),
                        )
                    nc.vector.tensor_scalar_add(
                        QT[o][:, hlf * 512:(hlf + 1) * 512], pq, bq_sb[:, o:o + 1]
                    )

            # ---- context transposes (reuse actT slots) ----
            ctxT = [
                p_actT.tile([128, SKV], f32r, name=f"ctxT{e}", tag="actT")
                for e in range(EI)
            ]
            for s in range(SKV // 128):
                cnat = p_nat.tile([128, E], f32, name="cnat", tag="nat")
                nc.sync.dma_start(out=cnat, in_=ctx[s * 128:(s + 1) * 128, :])
                for e in range(EI):
                    tps2 = ps_a.tile([128, 128], f32, name="tps2", tag="a")
                    nc.tensor.transpose(tps2, cnat[:, e * 128:(e + 1) * 128], ident)
                    nc.vector.tensor_copy(ctxT[e][:, s * 128:(s + 1) * 128], tps2)

            # ---- V natural [k, d] + bias, row-scaled by exp(mask), plus
            #      ones column per head for the softmax denominator ----
            Vg = [
                p_pers.tile([128, HPC * (D + 1)], f32r, name=f"Vg{k}")
                for k in range(KC)
            ]
            for k in range(KC):
                pvp = ps_a.tile([128, EO], f32, name="pvp", tag="a")
                for e in range(EI):
                    nc.tensor.matmul(
                        pvp,
                        lhsT=ctxT[e][:, k * 128:(k + 1) * 128],
                        rhs=wv_sb[e],
                        start=(e == 0),
                        stop=(e == EI - 1),
                    )
                vview = Vg[k].rearrange("p (h x) -> p h x", x=D + 1)
                nc.vector.tensor_add(
                    vview[:, :, 0:D],
                    pvp.rearrange("p (h d) -> p h d", d=D),
                    bv_bc.rearrange("p (h d) -> p h d", d=D),
                )
                nc.vector.tensor_scalar_mul(
                    vview[:, :, 0:D], vview[:, :, 0:D], wexp[:, k:k + 1]
                )
                nc.vector.tensor_copy(
                    vview[:, :, D:D + 1],
                    wexp[:, k:k + 1].broadcast_to([128, HPC, 1]),
                )

            # ---- per head-pair: K^T chunk, then attention ----
            KT = [p_pers.tile([128, SKV], f32r, name=f"KT{o}") for o in range(OC)]
            for pr in range(OC):
                for hlf in range(2):
                    pk = ps_a.tile([128, 512], f32, name="pk", tag="a")
                    for e in range(EI):
                        nc.tensor.matmul(
                            pk,
                            lhsT=wk_sb[e][:, pr * 128:(pr + 1) * 128],
                            rhs=ctxT[e][:, hlf * 512:(hlf + 1) * 512],
                            start=(e == 0),
                            stop=(e == EI - 1),
                        )
                    nc.vector.tensor_scalar_add(
                        KT[pr][:, hlf * 512:(hlf + 1) * 512], pk, bk_sb[:, pr:pr + 1]
                    )

                h0, h1 = 2 * pr, 2 * pr + 1
                for qh in range(2):
                    qs = slice(qh * 512, (qh + 1) * 512)
                    po0 = ps_pv.tile([D + 1, 512], f32, name="po0", tag="pv")
                    po1 = ps_pv.tile([D + 1, 512], f32, name="po1", tag="pv")
                    for k in range(KC):
                        ks = slice(k * 128, (k + 1) * 128)
                        pp = ps_s.tile([128, 1024], f32, name="pp", tag="s")
                        # S^T chunk for both heads, row-packed in the PE
                        nc.tensor.matmul(
                            pp[:, 0:512],
                            lhsT=KT[pr][0:64, ks],
                            rhs=QT[pr][0:64, qs],
                            start=True,
                            stop=True,
                        )
                        nc.tensor.matmul(
                            pp[:, 512:1024],
                            lhsT=KT[pr][64:128, ks],
                            rhs=QT[pr][64:128, qs],
                            start=True,
                            stop=True,
                        )
                        pt = p_pt.tile([128, 1024], f32r, name="pt", tag="pt")
                        nc.scalar.activation(pt, pp, AF.Exp, scale=0.125)
                        nc.tensor.matmul(
                            po0,
                            lhsT=Vg[k][:, h0 * (D + 1):(h0 + 1) * (D + 1)],
                            rhs=pt[:, 0:512],
                            start=(k == 0),
                            stop=(k == KC - 1),
                        )
                        nc.tensor.matmul(
                            po1,
                            lhsT=Vg[k][:, h1 * (D + 1):(h1 + 1) * (D + 1)],
                            rhs=pt[:, 512:1024],
                            start=(k == 0),
                            stop=(k == KC - 1),
                        )
                    ot0 = p_ot.tile([D + 1, 512], f32, name="ot0", tag="ot")
                    ot1 = p_ot.tile([D + 1, 512], f32, name="ot1", tag="ot")
                    nc.vector.tensor_copy(ot0, po0)
                    nc.vector.tensor_copy(ot1, po1)
                    for q4 in range(4):
                        qchunk = qh * 4 + q4
                        opt = p_out.tile([128, 128], f32, name="opt", tag="op")
                        for j, otj in enumerate((ot0, ot1)):
                            ptr = ps_a.tile([128, D + 1], f32, name="ptr", tag="a")
                            nc.tensor.transpose(
                                ptr,
                                otj[:, q4 * 128:(q4 + 1) * 128],
                                ident[0:D + 1, 0:D + 1],
                            )
                            rc = p_small.tile([128, 1], f32, name="rc", tag="rc")
                            nc.vector.reciprocal(rc, ptr[:, D:D + 1])
                            nc.vector.tensor_scalar_mul(
                                opt[:, j * D:(j + 1) * D], ptr[:, 0:D], rc
                            )
                        nc.sync.dma_start(
                            out=out[
                                qchunk * 128:(qchunk + 1) * 128,
                                pr * 128:(pr + 1) * 128,
                            ],
                            in_=opt,
                        )
    nc.finalize()
    return nc


def _get_nc():
    if "nc" not in _NC_CACHE:
        _NC_CACHE["nc"] = _build_nc()
    return _NC_CACHE["nc"]


def _make_in_maps(hidden_states, context, attention_mask, Wq, bq, Wk, bk, Wv, bv):
    hidden_states = np.asarray(hidden_states, dtype=np.float32)
    context = np.asarray(context, dtype=np.float32)
    attention_mask = np.asarray(attention_mask, dtype=np.float32)
    Wq, bq = np.asarray(Wq, np.float32), np.asarray(bq, np.float32)
    Wk, bk = np.asarray(Wk, np.float32), np.asarray(bk, np.float32)
    Wv, bv = np.asarray(Wv, np.float32), np.asarray(bv, np.float32)
    in_maps = []
    for c in range(N_CORES):
        b, g = c // 2, c % 2
        sl = slice(g * EO, (g + 1) * EO)
        in_maps.append({
            "hid": np.ascontiguousarray(hidden_states[b]),
            "ctx": np.ascontiguousarray(context[b]),
            "wq": np.ascontiguousarray(Wq[:, sl]),
            "wk": np.ascontiguousarray(Wk[:, sl]),
            "wv": np.ascontiguousarray(Wv[:, sl]),
            "bq": np.ascontiguousarray(bq[sl]),
            "bk": np.ascontiguousarray(bk[sl]),
            "bv": np.ascontiguousarray(bv[sl]),
            "mask": np.ascontiguousarray(attention_mask[b, 0, 0, :]),
        })
    return in_maps


def kernel(hidden_states, context, attention_mask, Wq, bq, Wk, bk, Wv, bv):
    global LAST_RESULTS
    from concourse.bass_utils import run_bass_kernel_spmd

    nc = _get_nc()
    in_maps = _make_in_maps(
        hidden_states, context, attention_mask, Wq, bq, Wk, bk, Wv, bv
    )
    res = run_bass_kernel_spmd(
        nc,
        in_maps,
        list(range(N_CORES)),
        trace=bool(os.environ.get("BASS_TRACE")),
    )
    LAST_RESULTS = res
    outp = np.empty((B, SQ, E), dtype=np.float32)
    for c in range(N_CORES):
        b, g = c // 2, c % 2
        outp[b][:, g * EO:(g + 1) * EO] = res.results[c]["out"]
    return outp


# revision 16
# speedup vs baseline: 1.0617x; 1.0617x over previous
"""Trainium2 Bass kernel for nn_Attention (B=4, S=1024, E=1024, H=16, D=64).

Sharding: 8 cores = 4 batches x 2 head-groups (8 heads each, host-side
slicing, no collectives). Per core: QKV projections in float32r on the PE
(activations PE-transposed on-chip so the contraction dim sits on
partitions), S^T = K^T-chunk^T-style matmuls with two heads row-packed in
the 128x128 array, exp on ACT with the 1/sqrt(D) scale folded in and the
additive attention mask folded into V-row scaling (exp(s+m) =
exp(s)*exp(m)), PV accumulation with an extra ones-column in V producing
the softmax denominator for free, PE-transpose back, per-partition
normalize, DMA out.
"""

import os
import sys

import numpy as np

for _p in ("/opt/trn_rl_repo",):
    if _p not in sys.path:
        sys.path.insert(0, _p)

B, SQ, SKV, E, H = 4, 1024, 1024, 1024, 16
D = 64
HPC = 8            # heads per core
EO = HPC * D       # 512 per-core projection output dim
N_CORES = 8
EI = E // 128      # 8 contraction chunks
KC = SKV // 128    # 8 key chunks
OC = EO // 128     # 4 output chunks = 4 head pairs

_NC_CACHE = {}
LAST_RESULTS = None


def _build_nc():
    import concourse.mybir as mybir
    from concourse import bacc
    from concourse.masks import make_identity
    from concourse.tile import TileContext

    f32 = mybir.dt.float32
    f32r = mybir.dt.float32r
    AF = mybir.ActivationFunctionType

    nc = bacc.Bacc("TRN2", target_bir_lowering=False, debug=False)
    hid = nc.declare_dram_parameter("hid", [SQ, E], f32, isOutput=False)
    ctx = nc.declare_dram_parameter("ctx", [SKV, E], f32, isOutput=False)
    wq = nc.declare_dram_parameter("wq", [E, EO], f32, isOutput=False)
    wk = nc.declare_dram_parameter("wk", [E, EO], f32, isOutput=False)
    wv = nc.declare_dram_parameter("wv", [E, EO], f32, isOutput=False)
    bq = nc.declare_dram_parameter("bq", [EO], f32, isOutput=False)
    bk = nc.declare_dram_parameter("bk", [EO], f32, isOutput=False)
    bv = nc.declare_dram_parameter("bv", [EO], f32, isOutput=False)
    mask = nc.declare_dram_parameter("mask", [SKV], f32, isOutput=False)
    out = nc.declare_dram_parameter("out", [SQ, EO], f32, isOutput=True)

    r = lambda ap: ap.bitcast(f32r)

    with TileContext(nc) as tc:
        with (
            tc.tile_pool(name="const", bufs=1) as p_const,
            tc.tile_pool(name="w", bufs=1) as p_w,
            tc.tile_pool(name="hidT", bufs=EI) as p_hidT,
            tc.tile_pool(name="ctxT", bufs=EI) as p_ctxT,
            tc.tile_pool(name="pers", bufs=1) as p_pers,
            tc.tile_pool(name="nat", bufs=4) as p_nat,
            tc.tile_pool(name="pt", bufs=3) as p_pt,
            tc.tile_pool(name="ot", bufs=4) as p_ot,
            tc.tile_pool(name="small", bufs=8) as p_small,
            tc.tile_pool(name="outp", bufs=3) as p_out,
            tc.tile_pool(name="ps_a", bufs=2, space="PSUM") as ps_a,
            tc.tile_pool(name="ps_s", bufs=2, space="PSUM") as ps_s,
            tc.tile_pool(name="ps_pv", bufs=2, space="PSUM") as ps_pv,
        ):
            ident = p_const.tile([128, 128], f32, name="ident")
            make_identity(nc, ident)

            bq_sb = p_const.tile([128, OC], f32, name="bq_sb")
            nc.gpsimd.dma_start(out=bq_sb, in_=bq.rearrange("(c p) -> p c", p=128))
            bk_sb = p_const.tile([128, OC], f32, name="bk_sb")
            nc.gpsimd.dma_start(out=bk_sb, in_=bk.rearrange("(c p) -> p c", p=128))
            bv_sb = p_const.tile([1, EO], f32, name="bv_sb")
            nc.gpsimd.dma_start(out=bv_sb, in_=bv[None, :])
            mask_sb = p_const.tile([128, KC], f32, name="mask_sb")
            nc.gpsimd.dma_start(out=mask_sb, in_=mask.rearrange("(c p) -> p c", p=128))
            wexp = p_const.tile([128, KC], f32, name="wexp")
            nc.scalar.activation(wexp, mask_sb, AF.Exp)
            ones1 = p_const.tile([1, 128], f32, name="ones1")
            nc.vector.memset(ones1, 1.0)

            # weights stream on SWDGE (Pool) in parallel with HWDGE (SP)
            wq_sb = []
            wv_sb = []
            wk_sb = []
            for e in range(EI):
                t = p_w.tile([128, EO], f32r, name=f"wq_sb{e}", tag=f"wq{e}")
                nc.gpsimd.dma_start(out=t, in_=r(wq[e * 128:(e + 1) * 128, :]))
                wq_sb.append(t)
            for e in range(EI):
                t = p_w.tile([128, EO], f32r, name=f"wv_sb{e}", tag=f"wv{e}")
                nc.gpsimd.dma_start(out=t, in_=r(wv[e * 128:(e + 1) * 128, :]))
                wv_sb.append(t)
            for e in range(EI):
                t = p_w.tile([128, EO], f32r, name=f"wk_sb{e}", tag=f"wk{e}")
                nc.gpsimd.dma_start(out=t, in_=r(wk[e * 128:(e + 1) * 128, :]))
                wk_sb.append(t)

            hidT = [
                p_hidT.tile([128, SQ], f32r, name=f"hidT{e}", tag="hidT")
                for e in range(EI)
            ]
            ctxT = [
                p_ctxT.tile([128, SKV], f32r, name=f"ctxT{e}", tag="ctxT")
                for e in range(EI)
            ]
            QT = [p_pers.tile([128, SQ], f32r, name=f"QT{o}") for o in range(OC)]
            KT = [p_pers.tile([128, SKV], f32r, name=f"KT{o}") for o in range(OC)]
            Vg = [
                p_pers.tile([128, HPC * (D + 1)], f32r, name=f"Vg{k}")
                for k in range(KC)
            ]

            def transpose_pair(dram, dst_tiles, sp, pool_nm):
                tiles = []
                for j in range(2):
                    s = 2 * sp + j
                    nat = p_nat.tile([128, E], f32, name=f"nat{pool_nm}", tag="nat")
                    nc.sync.dma_start(
                        out=nat[:, 0:E // 2],
                        in_=dram[s * 128:(s + 1) * 128, 0:E // 2],
                    )
                    nc.sync.dma_start(
                        out=nat[:, E // 2:E],
                        in_=dram[s * 128:(s + 1) * 128, E // 2:E],
                    )
                    tiles.append(nat)
                for e in range(EI):
                    tp = ps_a.tile([128, 256], f32, name="tp", tag="a")
                    for j in range(2):
                        nc.tensor.transpose(
                            tp[:, j * 128:(j + 1) * 128],
                            tiles[j][:, e * 128:(e + 1) * 128],
                            ident,
                        )
                    dst = dst_tiles[e][:, sp * 256:(sp + 1) * 256]
                    if e % 2 == 0:
                        nc.vector.tensor_copy(dst, tp)
                    else:
                        nc.scalar.activation(dst, tp, AF.Copy)

            def qt_half(o, hlf):
                pq = ps_a.tile([128, 512], f32, name="pq", tag="a")
                for e in range(EI):
                    nc.tensor.matmul(
                        pq,
                        lhsT=wq_sb[e][:, o * 128:(o + 1) * 128],
                        rhs=hidT[e][:, hlf * 512:(hlf + 1) * 512],
                        start=(e == 0),
                        stop=(e == EI - 1),
                    )
                nc.vector.tensor_scalar_add(
                    QT[o][:, hlf * 512:(hlf + 1) * 512], pq, bq_sb[:, o:o + 1]
                )

            def kt_half(o, hlf):
                pk = ps_a.tile([128, 512], f32, name="pk", tag="a")
                for e in range(EI):
                    nc.tensor.matmul(
                        pk,
                        lhsT=wk_sb[e][:, o * 128:(o + 1) * 128],
                        rhs=ctxT[e][:, hlf * 512:(hlf + 1) * 512],
                        start=(e == 0),
                        stop=(e == EI - 1),
                    )
                nc.vector.tensor_scalar_add(
                    KT[o][:, hlf * 512:(hlf + 1) * 512], pk, bk_sb[:, o:o + 1]
                )

            def v_chunk(k):
                pvp = ps_a.tile([128, EO], f32, name="pvp", tag="a")
                for e in range(EI):
                    nc.tensor.matmul(
                        pvp,
                        lhsT=ctxT[e][:, k * 128:(k + 1) * 128],
                        rhs=wv_sb[e],
                        start=(e == 0),
                        stop=(e == EI - 1),
                    )
                vview = Vg[k].rearrange("p (h x) -> p h x", x=D + 1)
                nc.vector.tensor_add(
                    vview[:, :, 0:D],
                    pvp.rearrange("p (h d) -> p h d", d=D),
                    bv_bc.rearrange("p (h d) -> p h d", d=D),
                )
                nc.vector.tensor_scalar_mul(
                    vview[:, :, 0:D], vview[:, :, 0:D], wexp[:, k:k + 1]
                )
                nc.vector.tensor_copy(
                    vview[:, :, D:D + 1],
                    wexp[:, k:k + 1].broadcast_to([128, HPC, 1]),
                )

            # ---- hidden stream with QT[0] halves interleaved ----
            transpose_pair(hid, hidT, 0, "h")
            transpose_pair(hid, hidT, 1, "h")
            qt_half(0, 0)
            transpose_pair(hid, hidT, 2, "h")
            transpose_pair(hid, hidT, 3, "h")
            qt_half(0, 1)

            # bv broadcast across partitions: ones1.T @ bv_sb
            bvps = ps_a.tile([128, EO], f32, name="bvps", tag="a")
            nc.tensor.matmul(bvps, lhsT=ones1, rhs=bv_sb, start=True, stop=True)
            bv_bc = p_const.tile([128, EO], f32, name="bv_bc")
            nc.vector.tensor_copy(bv_bc, bvps)

            # ---- context stream with V chunks and KT[0] halves interleaved ----
            transpose_pair(ctx, ctxT, 0, "c")
            v_chunk(0)
            v_chunk(1)
            transpose_pair(ctx, ctxT, 1, "c")
            v_chunk(2)
            v_chunk(3)
            kt_half(0, 0)
            transpose_pair(ctx, ctxT, 2, "c")
            v_chunk(4)
            v_chunk(5)
            transpose_pair(ctx, ctxT, 3, "c")
            v_chunk(6)
            v_chunk(7)
            kt_half(0, 1)

            # ---- per head-pair attention, with the NEXT pair's projection
            #      matmuls woven in (2 per kc slot) as PE filler while ACT
            #      computes exp, and PV lagging one kc behind exp ----
            def proj_steps(pr):
                steps = []
                for W, dstT, b_sb, rhsT in (
                    (wq_sb, QT, bq_sb, hidT),
                    (wk_sb, KT, bk_sb, ctxT),
                ):
                    for hlf in range(2):
                        state = {}
                        for e in range(EI):
                            def mk(e=e, W=W, dstT=dstT, b_sb=b_sb, rhsT=rhsT,
                                   hlf=hlf, state=state):
                                if e == 0:
                                    state["ps"] = ps_a.tile(
                                        [128, 512], f32, name="pj", tag="a"
                                    )
                                nc.tensor.matmul(
                                    state["ps"],
                                    lhsT=W[e][:, pr * 128:(pr + 1) * 128],
                                    rhs=rhsT[e][:, hlf * 512:(hlf + 1) * 512],
                                    start=(e == 0),
                                    stop=(e == EI - 1),
                                    skip_group_check=True,
                                )
                                if e == EI - 1:
                                    nc.vector.tensor_scalar_add(
                                        dstT[pr][:, hlf * 512:(hlf + 1) * 512],
                                        state["ps"],
                                        b_sb[:, pr:pr + 1],
                                    )
                            steps.append(mk)
                return steps

            for pr in range(OC):
                steps = proj_steps(pr + 1) if pr + 1 < OC else []
                si = 0
                h0, h1 = 2 * pr, 2 * pr + 1
                for qh in range(2):
                    qs = slice(qh * 512, (qh + 1) * 512)
                    po0 = ps_pv.tile([D + 1, 512], f32, name="po0", tag="pv")
                    po1 = ps_pv.tile([D + 1, 512], f32, name="po1", tag="pv")

                    def pv_pair(k, pt, po0=po0, po1=po1, h0=h0, h1=h1):
                        nc.tensor.matmul(
                            po0,
                            lhsT=Vg[k][:, h0 * (D + 1):(h0 + 1) * (D + 1)],
                            rhs=pt[:, 0:512],
                            start=(k == 0),
                            stop=(k == KC - 1),
                            skip_group_check=True,
                        )
                        nc.tensor.matmul(
                            po1,
                            lhsT=Vg[k][:, h1 * (D + 1):(h1 + 1) * (D + 1)],
                            rhs=pt[:, 512:1024],
                            start=(k == 0),
                            stop=(k == KC - 1),
                            skip_group_check=True,
                        )

                    pending = None
                    for k in range(KC):
                        ks = slice(k * 128, (k + 1) * 128)
                        pp = ps_s.tile([128, 1024], f32, name="pp", tag="s")
                        nc.tensor.matmul(
                            pp[:, 0:512],
                            lhsT=KT[pr][0:64, ks],
                            rhs=QT[pr][0:64, qs],
                            start=True,
                            stop=True,
                            skip_group_check=True,
                        )
                        nc.tensor.matmul(
                            pp[:, 512:1024],
                            lhsT=KT[pr][64:128, ks],
                            rhs=QT[pr][64:128, qs],
                            start=True,
                            stop=True,
                            skip_group_check=True,
                        )
                        pt = p_pt.tile([128, 1024], f32r, name="pt", tag="pt")
                        nc.scalar.activation(pt, pp, AF.Exp, scale=0.125)
                        for _ in range(2):
                            if si < len(steps):
                                steps[si]()
                                si += 1
                        if pending is not None:
                            pv_pair(*pending)
                        pending = (k, pt)
                    pv_pair(*pending)
                    ot0 = p_ot.tile([D + 1, 512], f32, name="ot0", tag="ot")
                    ot1 = p_ot.tile([D + 1, 512], f32, name="ot1", tag="ot")
                    nc.vector.tensor_copy(ot0, po0)
                    nc.vector.tensor_copy(ot1, po1)
                    # opt512 free layout: (q4, head j, d)
                    opt512 = p_out.tile([128, 512], f32, name="opt512", tag="op")
                    optv = opt512.rearrange("p (q4 j d) -> p q4 j d", q4=4, j=2)
                    for j, otj in enumerate((ot0, ot1)):
                        ptr4 = ps_a.tile([128, 4 * (D + 1)], f32, name="ptr4",
                                         tag="a")
                        for q4 in range(4):
                            nc.tensor.transpose(
                                ptr4[:, q4 * (D + 1):(q4 + 1) * (D + 1)],
                                otj[:, q4 * 128:(q4 + 1) * 128],
                                ident[0:D + 1, 0:D + 1],
                            )
                        ptrv = ptr4.rearrange("p (q4 x) -> p q4 x", x=D + 1)
                        rc4 = p_small.tile([128, 4], f32, name="rc4", tag="rc")
                        nc.vector.reciprocal(rc4, ptrv[:, :, D:D + 1])
                        nc.vector.tensor_mul(
                            optv[:, :, j, :],
                            ptrv[:, :, 0:D],
                            rc4[:, :, None].broadcast_to([128, 4, D]),
                        )
                    nc.sync.dma_start(
                        out=out[
                            qh * 512:(qh + 1) * 512, pr * 128:(pr + 1) * 128
                        ].rearrange("(q4 p) c -> p q4 c", p=128),
                        in_=opt512.rearrange("p (q4 c) -> p q4 c", q4=4),
                    )
    nc.finalize()
    return nc


def _get_nc():
    if "nc" not in _NC_CACHE:
        _NC_CACHE["nc"] = _build_nc()
    return _NC_CACHE["nc"]


def _make_in_maps(hidden_states, context, attention_mask, Wq, bq, Wk, bk, Wv, bv):
    hidden_states = np.asarray(hidden_states, dtype=np.float32)
    context = np.asarray(context, dtype=np.float32)
    attention_mask = np.asarray(attention_mask, dtype=np.float32)
    Wq, bq = np.asarray(Wq, np.float32), np.asarray(bq, np.float32)
    Wk, bk = np.asarray(Wk, np.float32), np.asarray(bk, np.float32)
    Wv, bv = np.asarray(Wv, np.float32), np.asarray(bv, np.float32)
    in_maps = []
    for c in range(N_CORES):
        b, g = c // 2, c % 2
        sl = slice(g * EO, (g + 1) * EO)
        in_maps.append({
            "hid": np.ascontiguousarray(hidden_states[b]),
            "ctx": np.ascontiguousarray(context[b]),
            "wq": np.ascontiguousarray(Wq[:, sl]),
            "wk": np.ascontiguousarray(Wk[:, sl]),
            "wv": np.ascontiguousarray(Wv[:, sl]),
            "bq": np.ascontiguousarray(bq[sl]),
            "bk": np.ascontiguousarray(bk[sl]),
            "bv": np.ascontiguousarray(bv[sl]),
            "mask": np.ascontiguousarray(attention_mask[b, 0, 0, :]),
        })
    return in_maps


def kernel(hidden_states, context, attention_mask, Wq, bq, Wk, bk, Wv, bv):
    global LAST_RESULTS
    from concourse.bass_utils import run_bass_kernel_spmd

    nc = _get_nc()
    in_maps = _make_in_maps(
        hidden_states, context, attention_mask, Wq, bq, Wk, bk, Wv, bv
    )
    res = run_bass_kernel_spmd(
        nc,
        in_maps,
        list(range(N_CORES)),
        trace=bool(os.environ.get("BASS_TRACE")),
    )
    LAST_RESULTS = res
    outp = np.empty((B, SQ, E), dtype=np.float32)
    for c in range(N_CORES):
        b, g = c // 2, c % 2
        outp[b][:, g * EO:(g + 1) * EO] = res.results[c]["out"]
    return outp
